# revision 1
# baseline (speedup 1.0000x reference)
"""Trainium2 Bass kernel for a dense cross-task transformer block.

Math notes
----------
The reference "attention" has sequence length 1 on the key axis, so
softmax(scores) == 1.0 exactly and the whole q/k/score path is dead:

    mha_len1(q_in, kv_in, ...) == (kv_in @ wv.T + bv) @ wo.T + bo

which folds (on host) into a single matmul with W = wo @ wv and
b = wo @ bv + bo.  The block is then:

    verb1 = LN(verb + noun @ W1.T + c1)          (ln_v)
    verb2 = verb1 + FFN_v(verb1)
    noun1 = LN(noun + verb2 @ W2.T + c2)         (ln_n)
    noun2 = noun1 + FFN_n(noun1)
    return verb2, noun2

Device strategy
---------------
Pure data parallel over 8 cores (batch 16384 -> 2048 rows/core), weights
replicated.  On device everything is kept feature-major ([E, batch]) so
every matmul contracts along the SBUF partition dim.  Matmuls run as
float32r (1 cycle/row for N>=256).  LayerNorm reduces across partitions
via ones-vector matmuls; stats are broadcast back across partitions with
K=1 matmuls.  The second FFN matmul runs in bf16 (hidden activations and
w2 weights) - the result only feeds a residual delta, so precision loss
is negligible.
"""

import numpy as np
import ml_dtypes
from contextlib import ExitStack

import concourse.bass as bass
import concourse.bacc as bacc_mod
import concourse.mybir as mybir
import concourse.tile as tile
from concourse.bass_utils import run_bass_kernel_spmd

E = 1024          # embed dim
H2 = 2048         # FFN hidden dim
B_TOTAL = 16384
NCORES = 8
B = B_TOTAL // NCORES   # 2048 rows per core
P = 128
EPS = 1e-5
CHUNK = 512       # attn/LN phase column chunk
NCHUNKS = B // CHUNK
KT = E // P       # 8  k-tiles over E
MT = E // P       # 8  m-tiles over E
HT = H2 // P      # 16 tiles over hidden

F32 = mybir.dt.float32
F32R = mybir.dt.float32r
BF16 = mybir.dt.bfloat16
AF = mybir.ActivationFunctionType
OP = mybir.AluOpType


def _load_pvec(nc, pool, dram_ap, ntiles, tag):
    """DRAM [ntiles*128] vector -> SBUF [128, ntiles], element (p,t) = v[t*128+p]."""
    t = pool.tile([P, ntiles], F32, tag=tag, name=tag)
    nc.sync.dma_start(out=t[:], in_=dram_ap.rearrange("(t p) -> p t", p=P))
    return t


def _build_program():
    nc = bacc_mod.Bacc("TRN2", target_bir_lowering=False)

    vT = nc.declare_dram_parameter("vT", [E, B], F32, isOutput=False)
    nT = nc.declare_dram_parameter("nT", [E, B], F32, isOutput=False)
    wvo1 = nc.declare_dram_parameter("wvo1", [E, E], F32, isOutput=False)     # (wo@wv).T : [k, m]
    bvo1 = nc.declare_dram_parameter("bvo1", [E], F32, isOutput=False)
    wvo2 = nc.declare_dram_parameter("wvo2", [E, E], F32, isOutput=False)
    bvo2 = nc.declare_dram_parameter("bvo2", [E], F32, isOutput=False)
    lnvg = nc.declare_dram_parameter("lnvg", [E], F32, isOutput=False)
    lnvb = nc.declare_dram_parameter("lnvb", [E], F32, isOutput=False)
    lnng = nc.declare_dram_parameter("lnng", [E], F32, isOutput=False)
    lnnb = nc.declare_dram_parameter("lnnb", [E], F32, isOutput=False)
    w1v = nc.declare_dram_parameter("w1v", [E, H2], F32, isOutput=False)      # fv_w1.T
    b1v = nc.declare_dram_parameter("b1v", [H2], F32, isOutput=False)
    w2v = nc.declare_dram_parameter("w2v", [H2, E], BF16, isOutput=False)     # fv_w2.T in bf16
    b2v = nc.declare_dram_parameter("b2v", [E], F32, isOutput=False)
    w1n = nc.declare_dram_parameter("w1n", [E, H2], F32, isOutput=False)
    b1n = nc.declare_dram_parameter("b1n", [H2], F32, isOutput=False)
    w2n = nc.declare_dram_parameter("w2n", [H2, E], BF16, isOutput=False)
    b2n = nc.declare_dram_parameter("b2n", [E], F32, isOutput=False)
    ones_d = nc.declare_dram_parameter("ones_d", [P, 1], F32, isOutput=False)
    verb_out = nc.declare_dram_parameter("verb_out", [E, B], F32, isOutput=True)
    noun_out = nc.declare_dram_parameter("noun_out", [E, B], F32, isOutput=True)

    with tile.TileContext(nc) as tc, ExitStack() as ctx:
        const = ctx.enter_context(tc.tile_pool(name="const", bufs=1))
        resid = ctx.enter_context(tc.tile_pool(name="resid", bufs=1))

        ones_col = const.tile([P, 1], F32R, tag="ones_col", name="ones_col")
        nc.sync.dma_start(out=ones_col[:], in_=ones_d[:, :].bitcast(F32R))
        ones_row = const.tile([1, P], F32, tag="ones_row", name="ones_row")
        nc.vector.memset(ones_row[:], 1.0)
        eps_t = const.tile([1, 1], F32, tag="eps", name="eps")
        nc.vector.memset(eps_t[:], EPS)

        bvo1_pb = _load_pvec(nc, const, bvo1[:], MT, "bvo1")
        bvo2_pb = _load_pvec(nc, const, bvo2[:], MT, "bvo2")
        lnvg_pb = _load_pvec(nc, const, lnvg[:], MT, "lnvg")
        lnvb_pb = _load_pvec(nc, const, lnvb[:], MT, "lnvb")
        lnng_pb = _load_pvec(nc, const, lnng[:], MT, "lnng")
        lnnb_pb = _load_pvec(nc, const, lnnb[:], MT, "lnnb")
        b1v_pb = _load_pvec(nc, const, b1v[:], HT, "b1v")
        b2v_pb = _load_pvec(nc, const, b2v[:], MT, "b2v")
        b1n_pb = _load_pvec(nc, const, b1n[:], HT, "b1n")
        b2n_pb = _load_pvec(nc, const, b2n[:], MT, "b2n")

        # persistent residual-stream tiles ([128, B] f32); verb1 in phases
        # A/B, overwritten as noun1 in phases C/D (same tags -> same slots)
        def resid_tiles():
            return [resid.tile([P, B], F32R, tag=f"r{m}", name=f"r{m}") for m in range(MT)]

        def attn_ln_phase(sfx, kxn_dram, res_dram, w_dram, bias_pb, g_pb, b_pb):
            """out_tiles[m][:, :] = LN(res + kxn.T @ w + bias) feature-major."""
            out_tiles = resid_tiles()
            with ExitStack() as pctx:
                wpool = pctx.enter_context(tc.tile_pool(name=f"wv{sfx}", bufs=1))
                kxp = pctx.enter_context(tc.tile_pool(name=f"kx{sfx}", bufs=1))
                vp = pctx.enter_context(tc.tile_pool(name=f"vp{sfx}", bufs=2))
                sqp = pctx.enter_context(tc.tile_pool(name=f"sq{sfx}", bufs=2))
                sm = pctx.enter_context(tc.tile_pool(name=f"sm{sfx}", bufs=1))
                aps = pctx.enter_context(
                    tc.tile_pool(name=f"aps{sfx}", bufs=2, space="PSUM"))
                stp = pctx.enter_context(
                    tc.tile_pool(name=f"st{sfx}", bufs=1, space="PSUM"))
                bcp = pctx.enter_context(
                    tc.tile_pool(name=f"bc{sfx}", bufs=1, space="PSUM"))

                w_tiles = []
                for k in range(KT):
                    wt = wpool.tile([P, E], F32R, tag=f"w{k}", name=f"w{k}")
                    nc.sync.dma_start(out=wt[:], in_=w_dram[k * P:(k + 1) * P, :].bitcast(F32R))
                    w_tiles.append(wt)

                for c in range(NCHUNKS):
                    cs = slice(c * CHUNK, (c + 1) * CHUNK)
                    kx = []
                    for k in range(KT):
                        t = kxp.tile([P, CHUNK], F32R, tag=f"k{k}", name=f"k{k}")
                        nc.sync.dma_start(out=t[:], in_=kxn_dram[k * P:(k + 1) * P, cs].bitcast(F32R))
                        kx.append(t)
                    stats_x = stp.tile([1, CHUNK], F32, tag="sx", name="sx")
                    stats_q = stp.tile([1, CHUNK], F32, tag="sq", name="sq")
                    for m in range(MT):
                        ps = aps.tile([P, CHUNK], F32, tag="ps", name="ps")
                        for k in range(KT):
                            nc.tensor.matmul(
                                ps[:],
                                lhsT=w_tiles[k][:, m * P:(m + 1) * P],
                                rhs=kx[k][:],
                                start=(k == 0), stop=(k == KT - 1))
                        vt = vp.tile([P, CHUNK], F32, tag="v", name="v")
                        nc.sync.dma_start(out=vt[:], in_=res_dram[m * P:(m + 1) * P, cs])
                        xt = out_tiles[m][:, cs]
                        nc.vector.tensor_add(xt, ps[:], vt[:])
                        nc.vector.tensor_scalar(
                            xt, xt, bias_pb[:, m:m + 1], None, OP.add)
                        sq = sqp.tile([P, CHUNK], F32R, tag="s", name="s")
                        nc.scalar.activation(sq[:], xt, AF.Square)
                        nc.tensor.matmul(stats_x[:], lhsT=ones_col[:],
                                         rhs=xt,
                                         start=(m == 0), stop=(m == MT - 1))
                        nc.tensor.matmul(stats_q[:], lhsT=ones_col[:],
                                         rhs=sq[:],
                                         start=(m == 0), stop=(m == MT - 1))
                    # column stats -> -mean, 1/std  ([1, CHUNK])
                    nm = sm.tile([1, CHUNK], F32, tag="nm", name="nm")
                    nc.scalar.activation(nm[:], stats_x[:], AF.Copy, scale=-1.0 / E)
                    t1 = sm.tile([1, CHUNK], F32, tag="t1", name="t1")
                    nc.scalar.activation(t1[:], stats_q[:], AF.Copy, scale=1.0 / E)
                    m2 = sm.tile([1, CHUNK], F32, tag="m2", name="m2")
                    nc.vector.tensor_mul(m2[:], nm[:], nm[:])
                    nc.vector.tensor_sub(t1[:], t1[:], m2[:])          # var
                    nc.scalar.activation(t1[:], t1[:], AF.Sqrt, bias=eps_t[:])
                    rs = sm.tile([1, CHUNK], F32, tag="rs", name="rs")
                    nc.vector.reciprocal(rs[:], t1[:])
                    # broadcast across partitions via K=1 matmuls (exact fp32)
                    nmB = bcp.tile([P, CHUNK], F32, tag="nmB", name="nmB")
                    nc.tensor.matmul(nmB[:], lhsT=ones_row[:], rhs=nm[:],
                                     start=True, stop=True)
                    rsB = bcp.tile([P, CHUNK], F32, tag="rsB", name="rsB")
                    nc.tensor.matmul(rsB[:], lhsT=ones_row[:], rhs=rs[:],
                                     start=True, stop=True)
                    for m in range(MT):
                        xt = out_tiles[m][:, cs]
                        nc.vector.tensor_add(xt, xt, nmB[:])
                        nc.vector.tensor_mul(xt, xt, rsB[:])
                        nc.vector.tensor_scalar(
                            xt, xt, g_pb[:, m:m + 1], b_pb[:, m:m + 1],
                            OP.mult, OP.add)
            return out_tiles

        def ffn_phase(sfx, in_tiles, h_tiles, w1_dram, b1_pb, w2_dram, b2_pb,
                      out_dram):
            """out = in + W2.T@gelu(W1.T@in + b1) + b2; streams to out_dram."""
            with ExitStack() as pctx:
                w1p = pctx.enter_context(tc.tile_pool(name=f"w1{sfx}", bufs=4))
                w2p = pctx.enter_context(tc.tile_pool(name=f"w2{sfx}", bufs=4))
                op = pctx.enter_context(tc.tile_pool(name=f"op{sfx}", bufs=2))
                fps = pctx.enter_context(
                    tc.tile_pool(name=f"fps{sfx}", bufs=2, space="PSUM"))
                for hm in range(HT):
                    ps = fps.tile([P, B], F32, tag="f", name="f")
                    for k in range(KT):
                        wt = w1p.tile([P, P], F32R, tag="w", name="w")
                        nc.sync.dma_start(
                            out=wt[:], in_=w1_dram[k * P:(k + 1) * P,
                                                   hm * P:(hm + 1) * P].bitcast(F32R))
                        for ns in range(B // 512):
                            nss = slice(ns * 512, (ns + 1) * 512)
                            nc.tensor.matmul(
                                ps[:, nss], lhsT=wt[:],
                                rhs=in_tiles[k][:, nss],
                                start=(k == 0), stop=(k == KT - 1))
                    nc.scalar.activation(h_tiles[hm][:], ps[:], AF.Gelu,
                                         bias=b1_pb[:, hm:hm + 1])
                for m in range(MT):
                    ps = fps.tile([P, B], F32, tag="f", name="f")
                    for k in range(HT):
                        wt = w2p.tile([P, P], BF16, tag="w", name="w")
                        nc.sync.dma_start(
                            out=wt[:], in_=w2_dram[k * P:(k + 1) * P,
                                                   m * P:(m + 1) * P])
                        for ns in range(B // 512):
                            nss = slice(ns * 512, (ns + 1) * 512)
                            nc.tensor.matmul(
                                ps[:, nss], lhsT=wt[:],
                                rhs=h_tiles[k][:, nss],
                                start=(k == 0), stop=(k == HT - 1))
                    ot = op.tile([P, B], F32, tag="o", name="o")
                    nc.vector.tensor_add(ot[:], ps[:], in_tiles[m][:])
                    nc.vector.tensor_scalar(
                        ot[:], ot[:], b2_pb[:, m:m + 1], None, OP.add)
                    nc.sync.dma_start(out=out_dram[m * P:(m + 1) * P, :], in_=ot[:])

        import os as _os
        _REP = int(_os.environ.get("BENCH_REPEAT", "1"))
        with ExitStack() as hctx:
            hp = hctx.enter_context(tc.tile_pool(name="hbf", bufs=1))

            def h_tiles():
                return [hp.tile([P, B], BF16, tag=f"h{i}", name=f"h{i}") for i in range(HT)]

            for _rep in range(_REP):
                # phase A: verb attends to noun, LN -> verb1 (resident)
                verb1 = attn_ln_phase(f"a{_rep}", nT, vT, wvo1, bvo1_pb,
                                      lnvg_pb, lnvb_pb)
                # phase B: verb FFN -> verb_out (DRAM)
                ffn_phase(f"b{_rep}", verb1, h_tiles(), w1v, b1v_pb, w2v,
                          b2v_pb, verb_out)
                # phase C: noun attends to verb2 (read back), LN -> noun1
                noun1 = attn_ln_phase(f"c{_rep}", verb_out, nT, wvo2, bvo2_pb,
                                      lnng_pb, lnnb_pb)
                # phase D: noun FFN -> noun_out
                ffn_phase(f"d{_rep}", noun1, h_tiles(), w1n, b1n_pb, w2n,
                          b2n_pb, noun_out)

    nc.finalize()
    return nc


_prog_cache = {}


def _get_program():
    if "nc" not in _prog_cache:
        _prog_cache["nc"] = _build_program()
    return _prog_cache["nc"]


def _prepare_maps(inputs):
    f32 = np.float32
    g = {k: np.asarray(v, f32) for k, v in inputs.items()}

    def fold(p):
        w = g[f"{p}_wo"] @ g[f"{p}_wv"]
        b = g[f"{p}_wo"] @ g[f"{p}_bv"] + g[f"{p}_bo"]
        return np.ascontiguousarray(w.T), np.ascontiguousarray(b)

    wvo1, bvo1 = fold("v2n")
    wvo2, bvo2 = fold("n2v")
    common = {
        "wvo1": wvo1, "bvo1": bvo1, "wvo2": wvo2, "bvo2": bvo2,
        "lnvg": g["ln_v_g"], "lnvb": g["ln_v_b"],
        "lnng": g["ln_n_g"], "lnnb": g["ln_n_b"],
        "w1v": np.ascontiguousarray(g["fv_w1"].T), "b1v": g["fv_b1"],
        "w2v": np.ascontiguousarray(g["fv_w2"].T).astype(ml_dtypes.bfloat16),
        "b2v": g["fv_b2"],
        "w1n": np.ascontiguousarray(g["fn_w1"].T), "b1n": g["fn_b1"],
        "w2n": np.ascontiguousarray(g["fn_w2"].T).astype(ml_dtypes.bfloat16),
        "b2n": g["fn_b2"],
        "ones_d": np.ones((128, 1), f32),
    }
    vT = np.ascontiguousarray(g["verb_features"].T)   # [E, 16384]
    nT = np.ascontiguousarray(g["noun_features"].T)
    in_maps = []
    for i in range(NCORES):
        cs = slice(i * B, (i + 1) * B)
        m = dict(common)
        m["vT"] = np.ascontiguousarray(vT[:, cs])
        m["nT"] = np.ascontiguousarray(nT[:, cs])
        in_maps.append(m)
    return in_maps


def kernel(**inputs):
    nc = _get_program()
    in_maps = _prepare_maps(inputs)
    res = run_bass_kernel_spmd(nc, in_maps, list(range(NCORES))).results
    verb = np.concatenate([res[i]["verb_out"] for i in range(NCORES)], axis=1)
    noun = np.concatenate([res[i]["noun_out"] for i in range(NCORES)], axis=1)
    return np.ascontiguousarray(verb.T), np.ascontiguousarray(noun.T)



# revision 6
# speedup vs baseline: 1.2064x; 1.2064x over previous
"""Trainium2 Bass kernel for a dense cross-task transformer block.

Math notes
----------
The reference "attention" has sequence length 1 on the key axis, so
softmax(scores) == 1.0 exactly and the whole q/k/score path is dead:

    mha_len1(q_in, kv_in, ...) == (kv_in @ wv.T + bv) @ wo.T + bo

which folds (on host) into a single matmul with W = wo @ wv and
b = wo @ bv + bo.  The block is then:

    verb1 = LN(verb + noun @ W1.T + c1)          (ln_v)
    verb2 = verb1 + FFN_v(verb1)
    noun1 = LN(noun + verb2 @ W2.T + c2)         (ln_n)
    noun2 = noun1 + FFN_n(noun1)
    return verb2, noun2

Device strategy
---------------
Pure data parallel over 8 cores (batch 16384 -> 2048 rows/core), weights
replicated.  Everything is feature-major ([E, batch]) so matmuls
contract along the SBUF partition dim.  All matmul operands are bf16
(fp32 PSUM accumulation); LN statistics are computed in fp32.

The kernel runs a single fused pipeline over 4 column chunks of 512:
for each chunk, stage A (verb<-noun attn + LN), B (verb FFN), C
(noun<-verb2 attn + LN, verb2 consumed straight from SBUF), D (noun
FFN).  Column chunks are independent end-to-end, so the Tile scheduler
overlaps chunk c+1's matmuls with chunk c's LN/evac tails, keeping the
PE dense (HAM stays at full clock).  noun tiles are loaded once per
chunk and serve both as stage-A rhs and stage-C residual.  LayerNorm
reduces across partitions with ones-vector matmuls; stats broadcast
back via K=1 matmuls.  Outputs are written bf16 and upcast on host.
"""

import numpy as np
import ml_dtypes
from contextlib import ExitStack

import concourse.bass as bass
import concourse.bacc as bacc_mod
import concourse.mybir as mybir
import concourse.tile as tile
from concourse.bass_utils import run_bass_kernel_spmd

E = 1024          # embed dim
H2 = 2048         # FFN hidden dim
B_TOTAL = 16384
NCORES = 8
B = B_TOTAL // NCORES   # 2048 rows per core
P = 128
EPS = 1e-5
CHUNK = 512
NCH = B // CHUNK  # 4
KT = E // P       # 8
MT = E // P       # 8
HT = H2 // P      # 16

F32 = mybir.dt.float32
BF16 = mybir.dt.bfloat16
AF = mybir.ActivationFunctionType
OP = mybir.AluOpType


def _load_pvec(nc, pool, dram_ap, ntiles, tag):
    """DRAM [ntiles*128] vector -> SBUF [128, ntiles], element (p,t) = v[t*128+p]."""
    t = pool.tile([P, ntiles], F32, tag=tag, name=tag)
    nc.sync.dma_start(out=t[:], in_=dram_ap.rearrange("(t p) -> p t", p=P))
    return t


def _build_program():
    nc = bacc_mod.Bacc("TRN2", target_bir_lowering=False)

    vT = nc.declare_dram_parameter("vT", [E, B], BF16, isOutput=False)
    nT = nc.declare_dram_parameter("nT", [E, B], BF16, isOutput=False)
    wvo1 = nc.declare_dram_parameter("wvo1", [E, E], BF16, isOutput=False)   # (wo@wv).T : [k, m]
    bvo1 = nc.declare_dram_parameter("bvo1", [E], F32, isOutput=False)
    wvo2 = nc.declare_dram_parameter("wvo2", [E, E], BF16, isOutput=False)
    bvo2 = nc.declare_dram_parameter("bvo2", [E], F32, isOutput=False)
    lnvg = nc.declare_dram_parameter("lnvg", [E], F32, isOutput=False)
    lnvb = nc.declare_dram_parameter("lnvb", [E], F32, isOutput=False)
    lnng = nc.declare_dram_parameter("lnng", [E], F32, isOutput=False)
    lnnb = nc.declare_dram_parameter("lnnb", [E], F32, isOutput=False)
    w1v = nc.declare_dram_parameter("w1v", [E, H2], BF16, isOutput=False)    # fv_w1.T
    b1v = nc.declare_dram_parameter("b1v", [H2], F32, isOutput=False)
    w2v = nc.declare_dram_parameter("w2v", [H2, E], BF16, isOutput=False)    # fv_w2.T
    b2v = nc.declare_dram_parameter("b2v", [E], F32, isOutput=False)
    w1n = nc.declare_dram_parameter("w1n", [E, H2], BF16, isOutput=False)
    b1n = nc.declare_dram_parameter("b1n", [H2], F32, isOutput=False)
    w2n = nc.declare_dram_parameter("w2n", [H2, E], BF16, isOutput=False)
    b2n = nc.declare_dram_parameter("b2n", [E], F32, isOutput=False)
    verb_out = nc.declare_dram_parameter("verb_out", [E, B], BF16, isOutput=True)
    noun_out = nc.declare_dram_parameter("noun_out", [E, B], BF16, isOutput=True)

    with tile.TileContext(nc) as tc, ExitStack() as ctx:
        const = ctx.enter_context(tc.tile_pool(name="const", bufs=1))
        wres = ctx.enter_context(tc.tile_pool(name="wres", bufs=1))
        w1p = ctx.enter_context(tc.tile_pool(name="w1p", bufs=1))
        w2p = ctx.enter_context(tc.tile_pool(name="w2p", bufs=1))
        # activation streams
        nounp = ctx.enter_context(tc.tile_pool(name="nounp", bufs=2))
        vresp = ctx.enter_context(tc.tile_pool(name="vresp", bufs=1))
        xp = ctx.enter_context(tc.tile_pool(name="xp", bufs=1))
        sqp = ctx.enter_context(tc.tile_pool(name="sqp", bufs=1))
        y1p = ctx.enter_context(tc.tile_pool(name="y1p", bufs=2))
        hp = ctx.enter_context(tc.tile_pool(name="hp", bufs=1))
        y2p = ctx.enter_context(tc.tile_pool(name="y2p", bufs=1))
        smp = ctx.enter_context(tc.tile_pool(name="smp", bufs=2))
        bbp = ctx.enter_context(tc.tile_pool(name="bbp", bufs=2))
        # PSUM pools
        mps = ctx.enter_context(tc.tile_pool(name="mps", bufs=3, space="PSUM"))
        sps = ctx.enter_context(tc.tile_pool(name="sps", bufs=1, space="PSUM"))
        bps = ctx.enter_context(tc.tile_pool(name="bps", bufs=1, space="PSUM"))
        wup = ctx.enter_context(tc.tile_pool(name="wup", bufs=1, space="PSUM"))

        # ---- PE warmup: dense matmuls with no DMA deps, trips HAM to 8/8
        warm_w = const.tile([P, P], BF16, tag="warm_w", name="warm_w")
        nc.vector.memset(warm_w[:], 1.0)
        warm_r = const.tile([P, 256], BF16, tag="warm_r", name="warm_r")
        nc.vector.memset(warm_r[:], 0.0)
        wps = wup.tile([P, 256], F32, tag="wps", name="wps")
        for i in range(40):
            nc.tensor.matmul(wps[:], lhsT=warm_w[:], rhs=warm_r[:],
                             start=(i == 0), stop=(i == 39))

        ones_col = const.tile([P, 1], BF16, tag="ones_col", name="ones_col")
        nc.vector.memset(ones_col[:], 1.0)
        ones_row = const.tile([1, P], BF16, tag="ones_row", name="ones_row")
        nc.vector.memset(ones_row[:], 1.0)
        ones_row_f = const.tile([1, P], F32, tag="ones_row_f", name="ones_row_f")
        nc.vector.memset(ones_row_f[:], 1.0)
        eps_t = const.tile([1, 1], F32, tag="eps", name="eps")
        nc.vector.memset(eps_t[:], EPS)

        bvo1_pb = _load_pvec(nc, const, bvo1[:], MT, "bvo1")
        bvo2_pb = _load_pvec(nc, const, bvo2[:], MT, "bvo2")
        lnvg_pb = _load_pvec(nc, const, lnvg[:], MT, "lnvg")
        lnvb_pb = _load_pvec(nc, const, lnvb[:], MT, "lnvb")
        lnng_pb = _load_pvec(nc, const, lnng[:], MT, "lnng")
        lnnb_pb = _load_pvec(nc, const, lnnb[:], MT, "lnnb")
        b1v_pb = _load_pvec(nc, const, b1v[:], HT, "b1v")
        b2v_pb = _load_pvec(nc, const, b2v[:], MT, "b2v")
        b1n_pb = _load_pvec(nc, const, b1n[:], HT, "b1n")
        b2n_pb = _load_pvec(nc, const, b2n[:], MT, "b2n")

        # resident attention weights (k-slabs [128, E])
        wvo1_t = []
        wvo2_t = []
        for k in range(KT):
            t1 = wres.tile([P, E], BF16, tag=f"wv1_{k}", name=f"wv1_{k}")
            nc.sync.dma_start(out=t1[:], in_=wvo1[k * P:(k + 1) * P, :])
            wvo1_t.append(t1)
            t2 = wres.tile([P, E], BF16, tag=f"wv2_{k}", name=f"wv2_{k}")
            nc.sync.dma_start(out=t2[:], in_=wvo2[k * P:(k + 1) * P, :])
            wvo2_t.append(t2)

        def attn_ln(sfx, kx_tiles, res_tiles, w_tiles, bias_pb, g_pb, b_pb):
            """return y1[m] (bf16 [128,CHUNK]) = LN(res + W.T@kx + bias)."""
            x_tiles = []
            sq_tiles = []
            for m in range(MT):
                ps = mps.tile([P, CHUNK], F32, tag="ps", name="ps")
                for k in range(KT):
                    nc.tensor.matmul(
                        ps[:], lhsT=w_tiles[k][:, m * P:(m + 1) * P],
                        rhs=kx_tiles[k][:],
                        start=(k == 0), stop=(k == KT - 1))
                xt = xp.tile([P, CHUNK], BF16, tag=f"x{m}", name=f"x{m}")
                nc.vector.tensor_add(xt[:], ps[:], res_tiles[m][:])
                nc.vector.tensor_scalar(
                    xt[:], xt[:], bias_pb[:, m:m + 1], None, OP.add)
                sq = sqp.tile([P, CHUNK], BF16, tag=f"s{m}", name=f"s{m}")
                nc.scalar.activation(sq[:], xt[:], AF.Square)
                x_tiles.append(xt)
                sq_tiles.append(sq)
            # stats matmuls AFTER all mains: keeps the in-order PE queue from
            # stalling on the DVE/ACT evacuations they depend on
            stats_x = sps.tile([1, CHUNK], F32, tag="sx", name="sx")
            stats_q = sps.tile([1, CHUNK], F32, tag="sq", name="sq")
            for m in range(MT):
                nc.tensor.matmul(stats_x[:], lhsT=ones_col[:],
                                 rhs=x_tiles[m][:],
                                 start=(m == 0), stop=(m == MT - 1))
                nc.tensor.matmul(stats_q[:], lhsT=ones_col[:],
                                 rhs=sq_tiles[m][:],
                                 start=(m == 0), stop=(m == MT - 1))
            # column stats -> -mean, 1/std  ([1, CHUNK])
            nm = smp.tile([1, CHUNK], F32, tag="nm", name="nm")
            nc.scalar.activation(nm[:], stats_x[:], AF.Copy, scale=-1.0 / E)
            t1 = smp.tile([1, CHUNK], F32, tag="t1", name="t1")
            nc.scalar.activation(t1[:], stats_q[:], AF.Copy, scale=1.0 / E)
            m2 = smp.tile([1, CHUNK], F32, tag="m2", name="m2")
            nc.vector.tensor_mul(m2[:], nm[:], nm[:])
            nc.vector.tensor_sub(t1[:], t1[:], m2[:])          # var
            nc.scalar.activation(t1[:], t1[:], AF.Sqrt, bias=eps_t[:])
            rs = smp.tile([1, CHUNK], F32, tag="rs", name="rs")
            nc.vector.reciprocal(rs[:], t1[:])
            nmh = smp.tile([1, CHUNK], BF16, tag="nmh", name="nmh")
            nc.vector.tensor_copy(nmh[:], nm[:])
            # broadcast across partitions via K=1 matmuls; 1/std stays fp32
            # (a bf16 rounding here is a common-mode per-column scale error
            # that propagates straight to the output)
            nmps = bps.tile([P, CHUNK], F32, tag="nmps", name="nmps")
            nc.tensor.matmul(nmps[:], lhsT=ones_row[:], rhs=nmh[:],
                             start=True, stop=True)
            rsps = bps.tile([P, CHUNK], F32, tag="rsps", name="rsps")
            nc.tensor.matmul(rsps[:], lhsT=ones_row_f[:], rhs=rs[:],
                             start=True, stop=True)
            nmB = bbp.tile([P, CHUNK], BF16, tag="nmB", name="nmB")
            nc.vector.tensor_copy(nmB[:], nmps[:])
            rsB = bbp.tile([P, CHUNK], F32, tag="rsB", name="rsB")
            nc.vector.tensor_copy(rsB[:], rsps[:])
            y_tiles = []
            for m in range(MT):
                yt = y1p.tile([P, CHUNK], BF16, tag=f"y{m}", name=f"y{m}")
                nc.vector.tensor_add(yt[:], x_tiles[m][:], nmB[:])
                nc.vector.tensor_mul(yt[:], yt[:], rsB[:])
                nc.vector.tensor_scalar(
                    yt[:], yt[:], g_pb[:, m:m + 1], b_pb[:, m:m + 1],
                    OP.mult, OP.add)
                y_tiles.append(yt)
            return y_tiles

        def ffn(sfx, y_tiles, w1_dram, b1_pb, w2_dram, b2_pb, out_dram, cs):
            """return y2[m] (bf16) = y + W2.T@gelu(W1.T@y + b1) + b2;
            also streams y2 to out_dram columns cs."""
            w1_t = []
            for k in range(KT):
                wt = w1p.tile([P, H2], BF16, tag=f"w1_{k}", name=f"w1_{k}")
                nc.sync.dma_start(out=wt[:], in_=w1_dram[k * P:(k + 1) * P, :])
                w1_t.append(wt)
            h_tiles = []
            for hm in range(HT):
                ps = mps.tile([P, CHUNK], F32, tag="ps", name="ps")
                for k in range(KT):
                    nc.tensor.matmul(
                        ps[:], lhsT=w1_t[k][:, hm * P:(hm + 1) * P],
                        rhs=y_tiles[k][:],
                        start=(k == 0), stop=(k == KT - 1))
                ht = hp.tile([P, CHUNK], BF16, tag=f"h{hm}", name=f"h{hm}")
                nc.scalar.activation(ht[:], ps[:], AF.Gelu,
                                     bias=b1_pb[:, hm:hm + 1])
                h_tiles.append(ht)
            w2_t = []
            for k in range(HT):
                wt = w2p.tile([P, E], BF16, tag=f"w2_{k}", name=f"w2_{k}")
                nc.sync.dma_start(out=wt[:], in_=w2_dram[k * P:(k + 1) * P, :])
                w2_t.append(wt)
            y2_tiles = []
            for m in range(MT):
                ps = mps.tile([P, CHUNK], F32, tag="ps", name="ps")
                for k in range(HT):
                    nc.tensor.matmul(
                        ps[:], lhsT=w2_t[k][:, m * P:(m + 1) * P],
                        rhs=h_tiles[k][:],
                        start=(k == 0), stop=(k == HT - 1))
                yt = y2p.tile([P, CHUNK], BF16, tag=f"o{m}", name=f"o{m}")
                nc.vector.tensor_add(yt[:], ps[:], y_tiles[m][:])
                nc.vector.tensor_scalar(
                    yt[:], yt[:], b2_pb[:, m:m + 1], None, OP.add)
                nc.sync.dma_start(out=out_dram[m * P:(m + 1) * P, cs],
                                  in_=yt[:])
                y2_tiles.append(yt)
            return y2_tiles

        import os as _os
        _REP = int(_os.environ.get("BENCH_REPEAT", "1"))
        for _rep in range(_REP):
            for c in range(NCH):
                cs = slice(c * CHUNK, (c + 1) * CHUNK)
                noun_t = []
                vres_t = []
                for k in range(KT):
                    t = nounp.tile([P, CHUNK], BF16, tag=f"n{k}", name=f"n{k}")
                    nc.sync.dma_start(out=t[:], in_=nT[k * P:(k + 1) * P, cs])
                    noun_t.append(t)
                    t = vresp.tile([P, CHUNK], BF16, tag=f"v{k}", name=f"v{k}")
                    nc.sync.dma_start(out=t[:], in_=vT[k * P:(k + 1) * P, cs])
                    vres_t.append(t)
                # A: verb attends to noun, LN -> verb1
                verb1 = attn_ln(f"a{_rep}_{c}", noun_t, vres_t, wvo1_t,
                                bvo1_pb, lnvg_pb, lnvb_pb)
                # B: verb FFN -> verb2 (SBUF + DRAM)
                verb2 = ffn(f"b{_rep}_{c}", verb1, w1v, b1v_pb, w2v, b2v_pb,
                            verb_out, cs)
                # C: noun attends to verb2 (from SBUF), LN -> noun1
                noun1 = attn_ln(f"c{_rep}_{c}", verb2, noun_t, wvo2_t,
                                bvo2_pb, lnng_pb, lnnb_pb)
                # D: noun FFN -> noun2
                ffn(f"d{_rep}_{c}", noun1, w1n, b1n_pb, w2n, b2n_pb,
                    noun_out, cs)

    nc.finalize()
    return nc


_prog_cache = {}


def _get_program():
    if "nc" not in _prog_cache:
        _prog_cache["nc"] = _build_program()
    return _prog_cache["nc"]


def _prepare_maps(inputs):
    f32 = np.float32
    bf16 = ml_dtypes.bfloat16
    g = {k: np.asarray(v, f32) for k, v in inputs.items()}

    def fold(p):
        w = g[f"{p}_wo"] @ g[f"{p}_wv"]
        b = g[f"{p}_wo"] @ g[f"{p}_bv"] + g[f"{p}_bo"]
        return np.ascontiguousarray(w.T).astype(bf16), np.ascontiguousarray(b)

    wvo1, bvo1 = fold("v2n")
    wvo2, bvo2 = fold("n2v")
    common = {
        "wvo1": wvo1, "bvo1": bvo1, "wvo2": wvo2, "bvo2": bvo2,
        "lnvg": g["ln_v_g"], "lnvb": g["ln_v_b"],
        "lnng": g["ln_n_g"], "lnnb": g["ln_n_b"],
        "w1v": np.ascontiguousarray(g["fv_w1"].T).astype(bf16),
        "b1v": g["fv_b1"],
        "w2v": np.ascontiguousarray(g["fv_w2"].T).astype(bf16),
        "b2v": g["fv_b2"],
        "w1n": np.ascontiguousarray(g["fn_w1"].T).astype(bf16),
        "b1n": g["fn_b1"],
        "w2n": np.ascontiguousarray(g["fn_w2"].T).astype(bf16),
        "b2n": g["fn_b2"],
    }
    vT = np.ascontiguousarray(g["verb_features"].T).astype(bf16)  # [E, 16384]
    nT = np.ascontiguousarray(g["noun_features"].T).astype(bf16)
    in_maps = []
    for i in range(NCORES):
        cs = slice(i * B, (i + 1) * B)
        m = dict(common)
        m["vT"] = np.ascontiguousarray(vT[:, cs])
        m["nT"] = np.ascontiguousarray(nT[:, cs])
        in_maps.append(m)
    return in_maps


def kernel(**inputs):
    nc = _get_program()
    in_maps = _prepare_maps(inputs)
    res = run_bass_kernel_spmd(nc, in_maps, list(range(NCORES))).results
    verb = np.concatenate([res[i]["verb_out"] for i in range(NCORES)], axis=1)
    noun = np.concatenate([res[i]["noun_out"] for i in range(NCORES)], axis=1)
    return (np.ascontiguousarray(verb.T).astype(np.float32),
            np.ascontiguousarray(noun.T).astype(np.float32))


# revision 18
# speedup vs baseline: 1.3488x; 1.1180x over previous
"""Trainium2 Bass kernel for a dense cross-task transformer block.

Math notes
----------
The reference "attention" has sequence length 1 on the key axis, so
softmax(scores) == 1.0 exactly and the whole q/k/score path is dead:

    mha_len1(q_in, kv_in, ...) == (kv_in @ wv.T + bv) @ wo.T + bo

which folds (on host) into a single matmul with W = wo @ wv and
b = wo @ bv + bo.  The block is then:

    verb1 = LN(verb + noun @ W1.T + c1)          (ln_v)
    verb2 = verb1 + FFN_v(verb1)
    noun1 = LN(noun + verb2 @ W2.T + c2)         (ln_n)
    noun2 = noun1 + FFN_n(noun1)
    return verb2, noun2

The LN shift beta folds into the FFN biases on host (b1' = b1 + w1@beta,
b2' = b2 + beta), so the device only applies (x - mean) * rstd * gamma.

Device strategy
---------------
Pure data parallel over 8 cores (batch 16384 -> 2048 rows/core), weights
replicated.  Feature-major layout ([E, batch]); all matmul operands are
fp16 (fp32 PSUM accumulation, fp32 LN statistics).

Single fused pipeline over 4 column chunks of 512: per chunk, stage A
(verb<-noun attn + LN), B (verb FFN), C (noun<-verb2 attn + LN, verb2
straight from SBUF), D (noun FFN).  Emission is software-pipelined so
the in-order PE never idles on a LayerNorm tail: D(c-1) covers A(c)'s
tail, and A(c+1)'s main matmuls are hoisted before C(c)'s tail.
LN stats use ones-vector matmuls (lag-2 interleaved with the mains);
rstd = Exp(-0.5*Ln(var+eps)) on ScalarE (both in one ACT table set, and
a dummy Ln after each gelu batch prefetches the set switch off the
critical path).  Outputs are fp16, upcast on host.
"""

import numpy as np
from contextlib import ExitStack

import concourse.bass as bass
import concourse.bacc as bacc_mod
import concourse.mybir as mybir
import concourse.tile as tile
from concourse.bass_utils import run_bass_kernel_spmd

E = 1024          # embed dim
H2 = 2048         # FFN hidden dim
B_TOTAL = 16384
NCORES = 8
B = B_TOTAL // NCORES   # 2048 rows per core
P = 128
EPS = 1e-5
CHUNK = 512
NCH = B // CHUNK  # 4
KT = E // P       # 8
MT = E // P       # 8
HT = H2 // P      # 16

F32 = mybir.dt.float32
F16 = mybir.dt.float16
AF = mybir.ActivationFunctionType
OP = mybir.AluOpType


def _load_pvec(nc, pool, dram_ap, ntiles, tag):
    """DRAM [ntiles*128] vector -> SBUF [128, ntiles], element (p,t) = v[t*128+p]."""
    t = pool.tile([P, ntiles], F32, tag=tag, name=tag)
    nc.sync.dma_start(out=t[:], in_=dram_ap.rearrange("(t p) -> p t", p=P))
    return t


def _build_program():
    nc = bacc_mod.Bacc("TRN2", target_bir_lowering=False)

    vT = nc.declare_dram_parameter("vT", [E, B], F16, isOutput=False)
    nT = nc.declare_dram_parameter("nT", [E, B], F16, isOutput=False)
    wvo1 = nc.declare_dram_parameter("wvo1", [E, E], F16, isOutput=False)   # (wo@wv).T : [k, m]
    bvo1 = nc.declare_dram_parameter("bvo1", [E], F32, isOutput=False)
    wvo2 = nc.declare_dram_parameter("wvo2", [E, E], F16, isOutput=False)
    bvo2 = nc.declare_dram_parameter("bvo2", [E], F32, isOutput=False)
    lnvg = nc.declare_dram_parameter("lnvg", [E], F32, isOutput=False)
    lnng = nc.declare_dram_parameter("lnng", [E], F32, isOutput=False)
    w1v = nc.declare_dram_parameter("w1v", [E, H2], F16, isOutput=False)    # fv_w1.T
    b1v = nc.declare_dram_parameter("b1v", [H2], F32, isOutput=False)
    w2v = nc.declare_dram_parameter("w2v", [H2, E], F16, isOutput=False)    # fv_w2.T
    b2v = nc.declare_dram_parameter("b2v", [E], F32, isOutput=False)
    w1n = nc.declare_dram_parameter("w1n", [E, H2], F16, isOutput=False)
    b1n = nc.declare_dram_parameter("b1n", [H2], F32, isOutput=False)
    w2n = nc.declare_dram_parameter("w2n", [H2, E], F16, isOutput=False)
    b2n = nc.declare_dram_parameter("b2n", [E], F32, isOutput=False)
    verb_out = nc.declare_dram_parameter("verb_out", [E, B], F16, isOutput=True)
    noun_out = nc.declare_dram_parameter("noun_out", [E, B], F16, isOutput=True)
    scratch = nc.declare_dram_parameter("scratch", [1, 1], F32, isOutput=False)

    with tile.TileContext(nc) as tc, ExitStack() as ctx:
        const = ctx.enter_context(tc.tile_pool(name="const", bufs=1))
        wvp = ctx.enter_context(tc.tile_pool(name="wvp", bufs=2))
        w1p = ctx.enter_context(tc.tile_pool(name="w1p", bufs=1))
        w2p = ctx.enter_context(tc.tile_pool(name="w2p", bufs=1))
        nounp = ctx.enter_context(tc.tile_pool(name="nounp", bufs=2))
        vresp = ctx.enter_context(tc.tile_pool(name="vresp", bufs=1))
        xp = ctx.enter_context(tc.tile_pool(name="xp", bufs=2))
        sqp = ctx.enter_context(tc.tile_pool(name="sqp", bufs=3))
        y1p = ctx.enter_context(tc.tile_pool(name="y1p", bufs=1))
        hp = ctx.enter_context(tc.tile_pool(name="hp", bufs=1))
        vo_p = ctx.enter_context(tc.tile_pool(name="vo_p", bufs=1))
        no_p = ctx.enter_context(tc.tile_pool(name="no_p", bufs=3))
        smp = ctx.enter_context(tc.tile_pool(name="smp", bufs=1))
        bbp = ctx.enter_context(tc.tile_pool(name="bbp", bufs=2))
        # PSUM pools
        mps = ctx.enter_context(tc.tile_pool(name="mps", bufs=3, space="PSUM"))
        sps = ctx.enter_context(tc.tile_pool(name="sps", bufs=1, space="PSUM"))
        bps = ctx.enter_context(tc.tile_pool(name="bps", bufs=1, space="PSUM"))
        wup = ctx.enter_context(tc.tile_pool(name="wup", bufs=1, space="PSUM"))

        # ---- PE warmup: dense matmuls with no DMA deps, trips HAM to 8/8
        warm_w = const.tile([P, P], F16, tag="warm_w", name="warm_w")
        nc.vector.memset(warm_w[:], 1.0)
        warm_r = const.tile([P, 256], F16, tag="warm_r", name="warm_r")
        nc.vector.memset(warm_r[:], 0.0)
        wps = wup.tile([P, 256], F32, tag="wps", name="wps")
        for i in range(40):
            nc.tensor.matmul(wps[:], lhsT=warm_w[:], rhs=warm_r[:],
                             start=(i == 0), stop=(i == 39))

        ones_col = const.tile([P, 1], F16, tag="ones_col", name="ones_col")
        nc.vector.memset(ones_col[:], 1.0)
        ones_row = const.tile([1, P], F16, tag="ones_row", name="ones_row")
        nc.vector.memset(ones_row[:], 1.0)
        ones_row_f = const.tile([1, P], F32, tag="ones_row_f", name="ones_row_f")
        nc.vector.memset(ones_row_f[:], 1.0)
        eps_t = const.tile([1, 1], F32, tag="eps", name="eps")
        nc.vector.memset(eps_t[:], EPS)
        dum = const.tile([1, 1], F32, tag="dum", name="dum")
        nc.vector.memset(dum[:], 1.0)

        bvo1_pb = _load_pvec(nc, const, bvo1[:], MT, "bvo1")
        bvo2_pb = _load_pvec(nc, const, bvo2[:], MT, "bvo2")
        lnvg_pb = _load_pvec(nc, const, lnvg[:], MT, "lnvg")
        lnng_pb = _load_pvec(nc, const, lnng[:], MT, "lnng")
        b1v_pb = _load_pvec(nc, const, b1v[:], HT, "b1v")
        b2v_pb = _load_pvec(nc, const, b2v[:], MT, "b2v")
        b1n_pb = _load_pvec(nc, const, b1n[:], HT, "b1n")
        b2n_pb = _load_pvec(nc, const, b2n[:], MT, "b2n")

        def dummy_ln():
            # touch Ln so walrus inserts the nat_log_exp table load HERE,
            # while the PE is busy with mains, not on the LN critical path.
            # Self-chained (and DMA'd out at the end) so DCE keeps it.
            # Ln(0*x + 1) == 0 stays finite for CoreSim's NaN check.
            nc.scalar.activation(dum[:], dum[:], AF.Ln, bias=1.0, scale=0.0)

        def load_w_slabs(pool, dram, n, width, tagpfx):
            ts = []
            for k in range(n):
                t = pool.tile([P, width], F16, tag=f"{tagpfx}{k}",
                              name=f"{tagpfx}{k}")
                nc.sync.dma_start(out=t[:], in_=dram[k * P:(k + 1) * P, :])
                ts.append(t)
            return ts

        def attn_mains(kx_tiles, res_tiles, w_dram, bias_pb):
            """mains + evac + squares + (lag-2) stats matmuls.
            returns (x_tiles, stats_x, stats_q)."""
            w_tiles = load_w_slabs(wvp, w_dram, KT, E, "wv")
            x_tiles = []
            sq_tiles = []
            stats_x = sps.tile([1, CHUNK], F32, tag="sx", name="sx")
            stats_q = sps.tile([1, CHUNK], F32, tag="sq", name="sq")

            def stats_mm(j):
                nc.tensor.matmul(stats_x[:], lhsT=ones_col[:],
                                 rhs=x_tiles[j][:],
                                 start=(j == 0), stop=(j == MT - 1))
                nc.tensor.matmul(stats_q[:], lhsT=ones_col[:],
                                 rhs=sq_tiles[j][:],
                                 start=(j == 0), stop=(j == MT - 1))

            for m in range(MT):
                ps = mps.tile([P, CHUNK], F32, tag="ps", name="ps")
                for k in range(KT):
                    nc.tensor.matmul(
                        ps[:], lhsT=w_tiles[k][:, m * P:(m + 1) * P],
                        rhs=kx_tiles[k][:],
                        start=(k == 0), stop=(k == KT - 1))
                xt = xp.tile([P, CHUNK], F16, tag=f"x{m}", name=f"x{m}")
                nc.vector.scalar_tensor_tensor(
                    xt[:], ps[:], bias_pb[:, m:m + 1], res_tiles[m][:],
                    OP.add, OP.add)
                sq = sqp.tile([P, CHUNK], F16, tag="s", name="s")
                nc.scalar.activation(sq[:], xt[:], AF.Square)
                x_tiles.append(xt)
                sq_tiles.append(sq)
                if m >= 2:
                    stats_mm(m - 2)
            stats_mm(MT - 2)
            stats_mm(MT - 1)
            dummy_ln()
            return x_tiles, stats_x, stats_q

        def ln_tail(x_tiles, stats_x, stats_q, g_pb):
            """-> y1[m] fp16 = (x - mean) * rstd * g   (beta folded on host)."""
            mu = smp.tile([1, CHUNK], F32, tag="mu", name="mu")
            nc.vector.tensor_scalar(mu[:], stats_x[:], 1.0 / E, None, OP.mult)
            m2 = smp.tile([1, CHUNK], F32, tag="m2", name="m2")
            nc.vector.tensor_mul(m2[:], mu[:], mu[:])
            var = smp.tile([1, CHUNK], F32, tag="var", name="var")
            nc.vector.scalar_tensor_tensor(
                var[:], stats_q[:], 1.0 / E, m2[:], OP.mult, OP.subtract)
            nmh = smp.tile([1, CHUNK], F16, tag="nmh", name="nmh")
            nc.vector.tensor_scalar(nmh[:], mu[:], -1.0, None, OP.mult)
            # rstd = exp(-0.5 * ln(var + eps)); Ln/Exp share one table set
            nc.scalar.activation(var[:], var[:], AF.Ln, bias=eps_t[:])
            rs = smp.tile([1, CHUNK], F32, tag="rs", name="rs")
            nc.scalar.activation(rs[:], var[:], AF.Exp, scale=-0.5)
            # broadcast across partitions via K=1 matmuls (rstd stays fp32)
            nmps = bps.tile([P, CHUNK], F32, tag="nmps", name="nmps")
            nc.tensor.matmul(nmps[:], lhsT=ones_row[:], rhs=nmh[:],
                             start=True, stop=True)
            rsps = bps.tile([P, CHUNK], F32, tag="rsps", name="rsps")
            nc.tensor.matmul(rsps[:], lhsT=ones_row_f[:], rhs=rs[:],
                             start=True, stop=True)
            nmB = bbp.tile([P, CHUNK], F16, tag="nmB", name="nmB")
            nc.vector.tensor_copy(nmB[:], nmps[:])
            rB = bbp.tile([P, CHUNK], F16, tag="rB", name="rB")
            nc.vector.tensor_copy(rB[:], rsps[:])
            y_tiles = []
            for m in range(MT):
                yt = y1p.tile([P, CHUNK], F16, tag=f"y{m}", name=f"y{m}")
                nc.vector.tensor_add(yt[:], x_tiles[m][:], nmB[:])
                nc.vector.scalar_tensor_tensor(
                    yt[:], yt[:], g_pb[:, m:m + 1], rB[:], OP.mult, OP.mult)
                y_tiles.append(yt)
            return y_tiles

        def ffn(y_tiles, w1_dram, b1_pb, w2_dram, b2_pb, out_dram, cs, opool,
                otag, per_m):
            """y2[m] fp16 = y + W2.T@gelu(W1.T@y + b1') + b2'; streams to
            out_dram[:, cs]."""
            w1_t = load_w_slabs(w1p, w1_dram, KT, H2, "w1_")
            h_tiles = []
            for hm in range(HT):
                ps = mps.tile([P, CHUNK], F32, tag="ps", name="ps")
                for k in range(KT):
                    nc.tensor.matmul(
                        ps[:], lhsT=w1_t[k][:, hm * P:(hm + 1) * P],
                        rhs=y_tiles[k][:],
                        start=(k == 0), stop=(k == KT - 1))
                ht = hp.tile([P, CHUNK], F16, tag=f"h{hm}", name=f"h{hm}")
                nc.scalar.activation(ht[:], ps[:], AF.Gelu,
                                     bias=b1_pb[:, hm:hm + 1])
                h_tiles.append(ht)
            w2_t = load_w_slabs(w2p, w2_dram, HT, E, "w2_")
            y2_tiles = []
            for m in range(MT):
                ps = mps.tile([P, CHUNK], F32, tag="ps", name="ps")
                for k in range(HT):
                    nc.tensor.matmul(
                        ps[:], lhsT=w2_t[k][:, m * P:(m + 1) * P],
                        rhs=h_tiles[k][:],
                        start=(k == 0), stop=(k == HT - 1))
                tg = f"{otag}{m}" if per_m else otag
                yt = opool.tile([P, CHUNK], F16, tag=tg, name=f"{otag}{m}")
                nc.vector.scalar_tensor_tensor(
                    yt[:], ps[:], b2_pb[:, m:m + 1], y_tiles[m][:],
                    OP.add, OP.add)
                nc.sync.dma_start(out=out_dram[m * P:(m + 1) * P, cs],
                                  in_=yt[:])
                y2_tiles.append(yt)
            dummy_ln()
            return y2_tiles

        def load_chunk_inputs(c):
            cs = slice(c * CHUNK, (c + 1) * CHUNK)
            noun_t = []
            vres_t = []
            for k in range(KT):
                t = nounp.tile([P, CHUNK], F16, tag=f"n{k}", name=f"n{k}")
                nc.sync.dma_start(out=t[:], in_=nT[k * P:(k + 1) * P, cs])
                noun_t.append(t)
                t = vresp.tile([P, CHUNK], F16, tag=f"v{k}", name=f"v{k}")
                nc.sync.dma_start(out=t[:], in_=vT[k * P:(k + 1) * P, cs])
                vres_t.append(t)
            return noun_t, vres_t

        import os as _os
        _REP = int(_os.environ.get("BENCH_REPEAT", "1"))
        for _rep in range(_REP):
            noun_t, vres_t = load_chunk_inputs(0)
            A_pend = attn_mains(noun_t, vres_t, wvo1, bvo1_pb)
            A_res = noun_t
            for c in range(NCH):
                cs = slice(c * CHUNK, (c + 1) * CHUNK)
                verb1 = ln_tail(*A_pend, lnvg_pb)
                verb2 = ffn(verb1, w1v, b1v_pb, w2v, b2v_pb, verb_out, cs,
                            vo_p, "vo", True)
                C_pend = attn_mains(verb2, A_res, wvo2, bvo2_pb)
                if c < NCH - 1:
                    noun_t, vres_t = load_chunk_inputs(c + 1)
                    nxt = attn_mains(noun_t, vres_t, wvo1, bvo1_pb)
                    nxt_res = noun_t
                noun1 = ln_tail(*C_pend, lnng_pb)
                ffn(noun1, w1n, b1n_pb, w2n, b2n_pb, noun_out, cs,
                    no_p, "no", False)
                if c < NCH - 1:
                    A_pend, A_res = nxt, nxt_res
        # keep the dummy-Ln chain live past DCE
        nc.sync.dma_start(out=scratch[:, :], in_=dum[:])

    nc.finalize()
    return nc


_prog_cache = {}


def _get_program():
    if "nc" not in _prog_cache:
        _prog_cache["nc"] = _build_program()
    return _prog_cache["nc"]


def _prepare_maps(inputs):
    f32 = np.float32
    f16 = np.float16
    g = {k: np.asarray(v, f32) for k, v in inputs.items()}

    def fold(p):
        w = g[f"{p}_wo"] @ g[f"{p}_wv"]
        b = g[f"{p}_wo"] @ g[f"{p}_bv"] + g[f"{p}_bo"]
        return np.ascontiguousarray(w.T).astype(f16), np.ascontiguousarray(b)

    wvo1, bvo1 = fold("v2n")
    wvo2, bvo2 = fold("n2v")
    common = {
        "wvo1": wvo1, "bvo1": bvo1, "wvo2": wvo2, "bvo2": bvo2,
        "lnvg": g["ln_v_g"], "lnng": g["ln_n_g"],
        "w1v": np.ascontiguousarray(g["fv_w1"].T).astype(f16),
        "b1v": g["fv_b1"] + g["fv_w1"] @ g["ln_v_b"],
        "w2v": np.ascontiguousarray(g["fv_w2"].T).astype(f16),
        "b2v": g["fv_b2"] + g["ln_v_b"],
        "w1n": np.ascontiguousarray(g["fn_w1"].T).astype(f16),
        "b1n": g["fn_b1"] + g["fn_w1"] @ g["ln_n_b"],
        "w2n": np.ascontiguousarray(g["fn_w2"].T).astype(f16),
        "b2n": g["fn_b2"] + g["ln_n_b"],
    }
    vT = np.ascontiguousarray(g["verb_features"].T).astype(f16)  # [E, 16384]
    nT = np.ascontiguousarray(g["noun_features"].T).astype(f16)
    in_maps = []
    for i in range(NCORES):
        cs = slice(i * B, (i + 1) * B)
        m = dict(common)
        m["vT"] = np.ascontiguousarray(vT[:, cs])
        m["nT"] = np.ascontiguousarray(nT[:, cs])
        m["scratch"] = np.zeros((1, 1), f32)
        in_maps.append(m)
    return in_maps


def kernel(**inputs):
    nc = _get_program()
    in_maps = _prepare_maps(inputs)
    res = run_bass_kernel_spmd(nc, in_maps, list(range(NCORES))).results
    verb = np.concatenate([res[i]["verb_out"] for i in range(NCORES)], axis=1)
    noun = np.concatenate([res[i]["noun_out"] for i in range(NCORES)], axis=1)
    return (np.ascontiguousarray(verb.T).astype(np.float32),
            np.ascontiguousarray(noun.T).astype(np.float32))


# revision 25
# speedup vs baseline: 1.3749x; 1.0193x over previous
"""Trainium2 Bass kernel for a dense cross-task transformer block.

Math notes
----------
The reference "attention" has sequence length 1 on the key axis, so
softmax(scores) == 1.0 exactly and the whole q/k/score path is dead:

    mha_len1(q_in, kv_in, ...) == (kv_in @ wv.T + bv) @ wo.T + bo

which folds (on host) into a single matmul with W = wo @ wv and
b = wo @ bv + bo.  The block is then:

    verb1 = LN(verb + noun @ W1.T + c1)          (ln_v)
    verb2 = verb1 + FFN_v(verb1)
    noun1 = LN(noun + verb2 @ W2.T + c2)         (ln_n)
    noun2 = noun1 + FFN_n(noun1)
    return verb2, noun2

The LN shift beta folds into the FFN biases on host (b1' = b1 + w1@beta,
b2' = b2 + beta), so the device only applies (x - mean) * rstd * gamma.

Device strategy
---------------
Pure data parallel over 8 cores (batch 16384 -> 2048 rows/core), weights
replicated.  Feature-major layout ([E, batch]); all matmul operands are
fp16 (fp32 PSUM accumulation, fp32 LN statistics).

Single fused pipeline over 4 column chunks of 512: per chunk, stage A
(verb<-noun attn + LN), B (verb FFN), C (noun<-verb2 attn + LN, verb2
straight from SBUF), D (noun FFN).  Emission is software-pipelined so
the in-order PE never idles on a LayerNorm tail: D(c-1) covers A(c)'s
tail, and A(c+1)'s main matmuls are hoisted before C(c)'s tail.
LN stats use ones-vector matmuls (lag-2 interleaved with the mains);
rstd = Exp(-0.5*Ln(var+eps)) on ScalarE (both in one ACT table set, and
a dummy Ln after each gelu batch prefetches the set switch off the
critical path).  Outputs are fp16, upcast on host.
"""

import numpy as np
from contextlib import ExitStack

import concourse.bass as bass
import concourse.bacc as bacc_mod
import concourse.mybir as mybir
import concourse.tile as tile
from concourse.bass_utils import run_bass_kernel_spmd

E = 1024          # embed dim
H2 = 2048         # FFN hidden dim
B_TOTAL = 16384
NCORES = 8
B = B_TOTAL // NCORES   # 2048 rows per core
P = 128
EPS = 1e-5
CHUNK = 512
NCH = B // CHUNK  # 4
KT = E // P       # 8
MT = E // P       # 8
HT = H2 // P      # 16

F32 = mybir.dt.float32
F16 = mybir.dt.float16
AF = mybir.ActivationFunctionType
OP = mybir.AluOpType


def _load_pvec(nc, pool, dram_ap, ntiles, tag):
    """DRAM [128, ntiles] (host-packed, contiguous) -> SBUF [128, ntiles]."""
    t = pool.tile([P, ntiles], F32, tag=tag, name=tag)
    nc.sync.dma_start(out=t[:], in_=dram_ap[:, :])
    return t


def _pack_pvec(v):
    """[ntiles*128] -> [128, ntiles] with element (p,t) = v[t*128+p]."""
    return np.ascontiguousarray(np.asarray(v, np.float32).reshape(-1, P).T)


def _build_program():
    nc = bacc_mod.Bacc("TRN2", target_bir_lowering=False)

    vT = nc.declare_dram_parameter("vT", [E, B], F16, isOutput=False)
    nT = nc.declare_dram_parameter("nT", [E, B], F16, isOutput=False)
    wvo1 = nc.declare_dram_parameter("wvo1", [E, E], F16, isOutput=False)   # (wo@wv).T : [k, m]
    bvo1 = nc.declare_dram_parameter("bvo1", [P, MT], F32, isOutput=False)
    wvo2 = nc.declare_dram_parameter("wvo2", [E, E], F16, isOutput=False)
    bvo2 = nc.declare_dram_parameter("bvo2", [P, MT], F32, isOutput=False)
    lnvg = nc.declare_dram_parameter("lnvg", [P, MT], F32, isOutput=False)
    lnng = nc.declare_dram_parameter("lnng", [P, MT], F32, isOutput=False)
    w1v = nc.declare_dram_parameter("w1v", [E, H2], F16, isOutput=False)    # fv_w1.T
    b1v = nc.declare_dram_parameter("b1v", [P, HT], F32, isOutput=False)
    w2v = nc.declare_dram_parameter("w2v", [H2, E], F16, isOutput=False)    # fv_w2.T
    b2v = nc.declare_dram_parameter("b2v", [P, MT], F32, isOutput=False)
    w1n = nc.declare_dram_parameter("w1n", [E, H2], F16, isOutput=False)
    b1n = nc.declare_dram_parameter("b1n", [P, HT], F32, isOutput=False)
    w2n = nc.declare_dram_parameter("w2n", [H2, E], F16, isOutput=False)
    b2n = nc.declare_dram_parameter("b2n", [P, MT], F32, isOutput=False)
    verb_out = nc.declare_dram_parameter("verb_out", [E, B], F16, isOutput=True)
    noun_out = nc.declare_dram_parameter("noun_out", [E, B], F16, isOutput=True)
    scratch = nc.declare_dram_parameter("scratch", [1, 1], F32, isOutput=False)

    with tile.TileContext(nc) as tc, ExitStack() as ctx:
        const = ctx.enter_context(tc.tile_pool(name="const", bufs=1))
        wvp = ctx.enter_context(tc.tile_pool(name="wvp", bufs=2))
        w1p = ctx.enter_context(tc.tile_pool(name="w1p", bufs=1))
        w2p = ctx.enter_context(tc.tile_pool(name="w2p", bufs=1))
        nounp = ctx.enter_context(tc.tile_pool(name="nounp", bufs=2))
        vresp = ctx.enter_context(tc.tile_pool(name="vresp", bufs=1))
        xp = ctx.enter_context(tc.tile_pool(name="xp", bufs=2))
        sqp = ctx.enter_context(tc.tile_pool(name="sqp", bufs=3))
        y1p = ctx.enter_context(tc.tile_pool(name="y1p", bufs=1))
        hp = ctx.enter_context(tc.tile_pool(name="hp", bufs=1))
        vo_p = ctx.enter_context(tc.tile_pool(name="vo_p", bufs=1))
        no_p = ctx.enter_context(tc.tile_pool(name="no_p", bufs=3))
        smp = ctx.enter_context(tc.tile_pool(name="smp", bufs=1))
        bbp = ctx.enter_context(tc.tile_pool(name="bbp", bufs=2))
        # PSUM pools
        mps = ctx.enter_context(tc.tile_pool(name="mps", bufs=3, space="PSUM"))
        sps = ctx.enter_context(tc.tile_pool(name="sps", bufs=1, space="PSUM"))
        bps = ctx.enter_context(tc.tile_pool(name="bps", bufs=1, space="PSUM"))
        wup = ctx.enter_context(tc.tile_pool(name="wup", bufs=1, space="PSUM"))

        # ---- PE warmup: dense matmuls with no DMA deps, trips HAM to 8/8
        warm_w = const.tile([P, P], F16, tag="warm_w", name="warm_w")
        nc.vector.memset(warm_w[:], 1.0)
        warm_r = const.tile([P, 256], F16, tag="warm_r", name="warm_r")
        nc.vector.memset(warm_r[:], 0.0)
        wps = wup.tile([P, 256], F32, tag="wps", name="wps")
        for i in range(40):
            nc.tensor.matmul(wps[:], lhsT=warm_w[:], rhs=warm_r[:],
                             start=(i == 0), stop=(i == 39))

        ones_col = const.tile([P, 1], F16, tag="ones_col", name="ones_col")
        nc.vector.memset(ones_col[:], 1.0)
        ones_row = const.tile([1, P], F16, tag="ones_row", name="ones_row")
        nc.vector.memset(ones_row[:], 1.0)
        ones_row_f = const.tile([1, P], F32, tag="ones_row_f", name="ones_row_f")
        nc.vector.memset(ones_row_f[:], 1.0)
        eps_t = const.tile([1, 1], F32, tag="eps", name="eps")
        nc.vector.memset(eps_t[:], EPS)
        dum = const.tile([1, 1], F32, tag="dum", name="dum")
        nc.vector.memset(dum[:], 1.0)

        def dummy_ln():
            # touch Ln so walrus inserts the nat_log_exp table load HERE,
            # while the PE is busy with mains, not on the LN critical path.
            # Self-chained (and DMA'd out at the end) so DCE keeps it.
            # Ln(0*x + 1) == 0 stays finite for CoreSim's NaN check.
            nc.scalar.activation(dum[:], dum[:], AF.Ln, bias=1.0, scale=0.0)

        def load_w_slabs(pool, dram, n, width, tagpfx):
            ts = []
            for k in range(n):
                t = pool.tile([P, width], F16, tag=f"{tagpfx}{k}",
                              name=f"{tagpfx}{k}")
                nc.sync.dma_start(out=t[:], in_=dram[k * P:(k + 1) * P, :])
                ts.append(t)
            return ts

        def attn_mains(kx_tiles, res_tiles, w_tiles, bias_pb):
            """mains + evac + squares + (lag-2) stats matmuls.
            returns (x_tiles, stats_x, stats_q)."""
            x_tiles = []
            sq_tiles = []
            stats_x = sps.tile([1, CHUNK], F32, tag="sx", name="sx")
            stats_q = sps.tile([1, CHUNK], F32, tag="sq", name="sq")

            def stats_mm(j):
                nc.tensor.matmul(stats_x[:], lhsT=ones_col[:],
                                 rhs=x_tiles[j][:],
                                 start=(j == 0), stop=(j == MT - 1))
                nc.tensor.matmul(stats_q[:], lhsT=ones_col[:],
                                 rhs=sq_tiles[j][:],
                                 start=(j == 0), stop=(j == MT - 1))

            for m in range(MT):
                ps = mps.tile([P, CHUNK], F32, tag="ps", name="ps")
                for k in range(KT):
                    nc.tensor.matmul(
                        ps[:], lhsT=w_tiles[k][:, m * P:(m + 1) * P],
                        rhs=kx_tiles[k][:],
                        start=(k == 0), stop=(k == KT - 1))
                xt = xp.tile([P, CHUNK], F16, tag=f"x{m}", name=f"x{m}")
                nc.vector.scalar_tensor_tensor(
                    xt[:], ps[:], bias_pb[:, m:m + 1], res_tiles[m][:],
                    OP.add, OP.add)
                sq = sqp.tile([P, CHUNK], F16, tag="s", name="s")
                nc.scalar.activation(sq[:], xt[:], AF.Square)
                x_tiles.append(xt)
                sq_tiles.append(sq)
                if m >= 2:
                    stats_mm(m - 2)
            stats_mm(MT - 2)
            stats_mm(MT - 1)
            dummy_ln()
            return x_tiles, stats_x, stats_q

        def ln_tail(x_tiles, stats_x, stats_q, g_pb):
            """-> y1[m] fp16 = (x - mean) * rstd * g   (beta folded on host)."""
            mu = smp.tile([1, CHUNK], F32, tag="mu", name="mu")
            nc.vector.tensor_scalar(mu[:], stats_x[:], 1.0 / E, None, OP.mult)
            m2 = smp.tile([1, CHUNK], F32, tag="m2", name="m2")
            nc.vector.tensor_mul(m2[:], mu[:], mu[:])
            var = smp.tile([1, CHUNK], F32, tag="var", name="var")
            nc.vector.scalar_tensor_tensor(
                var[:], stats_q[:], 1.0 / E, m2[:], OP.mult, OP.subtract)
            nmh = smp.tile([1, CHUNK], F16, tag="nmh", name="nmh")
            nc.vector.tensor_scalar(nmh[:], mu[:], -1.0, None, OP.mult)
            # rstd = exp(-0.5 * ln(var + eps)); Ln/Exp share one table set
            nc.scalar.activation(var[:], var[:], AF.Ln, bias=eps_t[:])
            rs = smp.tile([1, CHUNK], F32, tag="rs", name="rs")
            nc.scalar.activation(rs[:], var[:], AF.Exp, scale=-0.5)
            # broadcast across partitions via K=1 matmuls (rstd stays fp32)
            nmps = bps.tile([P, CHUNK], F32, tag="nmps", name="nmps")
            nc.tensor.matmul(nmps[:], lhsT=ones_row[:], rhs=nmh[:],
                             start=True, stop=True)
            rsps = bps.tile([P, CHUNK], F32, tag="rsps", name="rsps")
            nc.tensor.matmul(rsps[:], lhsT=ones_row_f[:], rhs=rs[:],
                             start=True, stop=True)
            nmB = bbp.tile([P, CHUNK], F16, tag="nmB", name="nmB")
            nc.vector.tensor_copy(nmB[:], nmps[:])
            rB = bbp.tile([P, CHUNK], F16, tag="rB", name="rB")
            nc.vector.tensor_copy(rB[:], rsps[:])
            y_tiles = []
            for m in range(MT):
                yt = y1p.tile([P, CHUNK], F16, tag=f"y{m}", name=f"y{m}")
                nc.vector.tensor_add(yt[:], x_tiles[m][:], nmB[:])
                nc.vector.scalar_tensor_tensor(
                    yt[:], yt[:], g_pb[:, m:m + 1], rB[:], OP.mult, OP.mult)
                y_tiles.append(yt)
            return y_tiles

        def ffn(y_tiles, w1_dram, b1_pb, w2_dram, b2_pb, out_dram, cs, opool,
                otag, per_m):
            """y2[m] fp16 = y + W2.T@gelu(W1.T@y + b1') + b2'; streams to
            out_dram[:, cs]."""
            w1_t = load_w_slabs(w1p, w1_dram, KT, H2, "w1_")
            h_tiles = []
            for hm in range(HT):
                ps = mps.tile([P, CHUNK], F32, tag="ps", name="ps")
                for k in range(KT):
                    nc.tensor.matmul(
                        ps[:], lhsT=w1_t[k][:, hm * P:(hm + 1) * P],
                        rhs=y_tiles[k][:],
                        start=(k == 0), stop=(k == KT - 1))
                ht = hp.tile([P, CHUNK], F16, tag=f"h{hm}", name=f"h{hm}")
                nc.scalar.activation(ht[:], ps[:], AF.Gelu,
                                     bias=b1_pb[:, hm:hm + 1])
                h_tiles.append(ht)
            w2_t = load_w_slabs(w2p, w2_dram, HT, E, "w2_")
            y2_tiles = []
            for m in range(MT):
                ps = mps.tile([P, CHUNK], F32, tag="ps", name="ps")
                for k in range(HT):
                    nc.tensor.matmul(
                        ps[:], lhsT=w2_t[k][:, m * P:(m + 1) * P],
                        rhs=h_tiles[k][:],
                        start=(k == 0), stop=(k == HT - 1))
                tg = f"{otag}{m}" if per_m else otag
                yt = opool.tile([P, CHUNK], F16, tag=tg, name=f"{otag}{m}")
                nc.vector.scalar_tensor_tensor(
                    yt[:], ps[:], b2_pb[:, m:m + 1], y_tiles[m][:],
                    OP.add, OP.add)
                nc.sync.dma_start(out=out_dram[m * P:(m + 1) * P, cs],
                                  in_=yt[:])
                y2_tiles.append(yt)
            dummy_ln()
            return y2_tiles

        def load_chunk_inputs(c):
            cs = slice(c * CHUNK, (c + 1) * CHUNK)
            noun_t = []
            vres_t = []
            for k in range(KT):
                t = nounp.tile([P, CHUNK], F16, tag=f"n{k}", name=f"n{k}")
                nc.sync.dma_start(out=t[:], in_=nT[k * P:(k + 1) * P, cs])
                noun_t.append(t)
                t = vresp.tile([P, CHUNK], F16, tag=f"v{k}", name=f"v{k}")
                nc.sync.dma_start(out=t[:], in_=vT[k * P:(k + 1) * P, cs])
                vres_t.append(t)
            return noun_t, vres_t

        import os as _os
        _REP = int(_os.environ.get("BENCH_REPEAT", "1"))
        # chunk-0 critical-path DMAs lead the queue; bias vectors follow
        noun_0, vres_0 = load_chunk_inputs(0)
        wv_0 = load_w_slabs(wvp, wvo1, KT, E, "wv")
        bvo1_pb = _load_pvec(nc, const, bvo1, MT, "bvo1")
        bvo2_pb = _load_pvec(nc, const, bvo2, MT, "bvo2")
        lnvg_pb = _load_pvec(nc, const, lnvg, MT, "lnvg")
        lnng_pb = _load_pvec(nc, const, lnng, MT, "lnng")
        b1v_pb = _load_pvec(nc, const, b1v, HT, "b1v")
        b2v_pb = _load_pvec(nc, const, b2v, MT, "b2v")
        b1n_pb = _load_pvec(nc, const, b1n, HT, "b1n")
        b2n_pb = _load_pvec(nc, const, b2n, MT, "b2n")
        for _rep in range(_REP):
            if _rep == 0:
                noun_t, vres_t, wv_t = noun_0, vres_0, wv_0
            else:
                noun_t, vres_t = load_chunk_inputs(0)
                wv_t = load_w_slabs(wvp, wvo1, KT, E, "wv")
            A_pend = attn_mains(noun_t, vres_t, wv_t, bvo1_pb)
            A_res = noun_t
            for c in range(NCH):
                cs = slice(c * CHUNK, (c + 1) * CHUNK)
                verb1 = ln_tail(*A_pend, lnvg_pb)
                verb2 = ffn(verb1, w1v, b1v_pb, w2v, b2v_pb, verb_out, cs,
                            vo_p, "vo", True)
                wv_t = load_w_slabs(wvp, wvo2, KT, E, "wv")
                C_pend = attn_mains(verb2, A_res, wv_t, bvo2_pb)
                if c < NCH - 1:
                    noun_t, vres_t = load_chunk_inputs(c + 1)
                    wv_t = load_w_slabs(wvp, wvo1, KT, E, "wv")
                    nxt = attn_mains(noun_t, vres_t, wv_t, bvo1_pb)
                    nxt_res = noun_t
                noun1 = ln_tail(*C_pend, lnng_pb)
                ffn(noun1, w1n, b1n_pb, w2n, b2n_pb, noun_out, cs,
                    no_p, "no", False)
                if c < NCH - 1:
                    A_pend, A_res = nxt, nxt_res
        # keep the dummy-Ln chain live past DCE
        nc.sync.dma_start(out=scratch[:, :], in_=dum[:])

    nc.finalize()
    return nc


_prog_cache = {}


def _get_program():
    if "nc" not in _prog_cache:
        _prog_cache["nc"] = _build_program()
    return _prog_cache["nc"]


def _prepare_maps(inputs):
    f32 = np.float32
    f16 = np.float16
    g = {k: np.asarray(v, f32) for k, v in inputs.items()}

    def fold(p):
        w = g[f"{p}_wo"] @ g[f"{p}_wv"]
        b = g[f"{p}_wo"] @ g[f"{p}_bv"] + g[f"{p}_bo"]
        return np.ascontiguousarray(w.T).astype(f16), np.ascontiguousarray(b)

    wvo1, bvo1 = fold("v2n")
    wvo2, bvo2 = fold("n2v")
    common = {
        "wvo1": wvo1, "bvo1": _pack_pvec(bvo1),
        "wvo2": wvo2, "bvo2": _pack_pvec(bvo2),
        "lnvg": _pack_pvec(g["ln_v_g"]), "lnng": _pack_pvec(g["ln_n_g"]),
        "w1v": np.ascontiguousarray(g["fv_w1"].T).astype(f16),
        "b1v": _pack_pvec(g["fv_b1"] + g["fv_w1"] @ g["ln_v_b"]),
        "w2v": np.ascontiguousarray(g["fv_w2"].T).astype(f16),
        "b2v": _pack_pvec(g["fv_b2"] + g["ln_v_b"]),
        "w1n": np.ascontiguousarray(g["fn_w1"].T).astype(f16),
        "b1n": _pack_pvec(g["fn_b1"] + g["fn_w1"] @ g["ln_n_b"]),
        "w2n": np.ascontiguousarray(g["fn_w2"].T).astype(f16),
        "b2n": _pack_pvec(g["fn_b2"] + g["ln_n_b"]),
    }
    vT = np.ascontiguousarray(g["verb_features"].T).astype(f16)  # [E, 16384]
    nT = np.ascontiguousarray(g["noun_features"].T).astype(f16)
    in_maps = []
    for i in range(NCORES):
        cs = slice(i * B, (i + 1) * B)
        m = dict(common)
        m["vT"] = np.ascontiguousarray(vT[:, cs])
        m["nT"] = np.ascontiguousarray(nT[:, cs])
        m["scratch"] = np.zeros((1, 1), f32)
        in_maps.append(m)
    return in_maps


def kernel(**inputs):
    nc = _get_program()
    in_maps = _prepare_maps(inputs)
    res = run_bass_kernel_spmd(nc, in_maps, list(range(NCORES))).results
    verb = np.concatenate([res[i]["verb_out"] for i in range(NCORES)], axis=1)
    noun = np.concatenate([res[i]["noun_out"] for i in range(NCORES)], axis=1)
    return (np.ascontiguousarray(verb.T).astype(np.float32),
            np.ascontiguousarray(noun.T).astype(np.float32))
